# revision 7
# baseline (speedup 1.0000x reference)
"""MD-RNN (4-direction 2D GRU) Trainium2 kernel, fp8 DoubleRow edition.

Sharding: 8-way data-parallel over batch (B=256 -> 32 per core); each core runs
all 4 directional 2D-GRU scans as anti-diagonal wavefronts, interleaved so all
engines stay busy.

Layout (hidden-on-partition, "transposed"):
  - hidden states h^T: (128 partitions = hidden chunk kc in {0,1}, cells*B free)
    stored twice: fp8e4m3 (matmul operand) and bf16 (skip-path + head).
  - all matmuls are fp8 DoubleRow (2 stacked K=128 k-tiles per instruction at
    0.5 cycles/row): per 8-cell chunk, 18 DR instructions produce psum tiles
    P_r, P_z (input+bias+recurrence), P_nh (recurrence only), P_xn (input).
  - gate math: sigmoid/tanh on Act; t1=r*nh, t2=t1+xn, d=0.5*s-n, e=z*d,
    h'Bf=n+e on DVE; s=h+h2 and h'F8=n+e on Pool.

The patch tensor (im2col of x + bias ones row, split into two 9-row halves for
DoubleRow) is built host-side and streamed per-chunk from DRAM.
"""

import numpy as np
import ml_dtypes

GRID = 4
N_IMG = 32
S = N_IMG - (GRID - 1)          # 29 patch positions per axis
B_FULL = 256
N_CORES = 8
B = B_FULL // N_CORES           # 32 batch per core
H = 256
H3 = 3 * H                      # 768
OUT_DIM = 10
K_IN = GRID * GRID + 1          # 16 patch elems + ones row (bias trick)
KH = 9                          # DoubleRow half-height of the input projection

FWD = list(range(S))                 # 29 entries
BWD = list(range(S - 2, -1, -1))     # 28 entries (reference off-by-one kept)
DIRS = [(FWD, FWD), (BWD, FWD), (FWD, BWD), (BWD, BWD)]

CELLS_PER_CHUNK = 8             # 8 cells * B=32 = 256 cols; 4 psum banks/chunk
REPEAT = 1                      # body repetitions (timing calibration only)

F8 = ml_dtypes.float8_e4m3
BF = ml_dtypes.bfloat16


def _diag_infos():
    """Per direction: list over diagonals of (i_lo, i_hi, global cell base)."""
    infos = []
    base = 0
    for (yi, xi) in DIRS:
        ny, nx = len(yi), len(xi)
        diags = []
        for d in range(ny + nx - 1):
            ilo = max(0, d - (nx - 1))
            ihi = min(d, ny - 1)
            diags.append((ilo, ihi, base))
            base += ihi - ilo + 1
        infos.append(diags)
    return infos, base


DIAG_INFOS, TOT_CELLS = _diag_infos()


def _scan_index_arrays():
    """Image-space (y, x) of every cell in pt order (dir-major, diag-major)."""
    ys, xs = [], []
    for a, (yi, xi) in enumerate(DIRS):
        for d, (ilo, ihi, _) in enumerate(DIAG_INFOS[a]):
            for i in range(ilo, ihi + 1):
                ys.append(yi[i])
                xs.append(xi[d - i])
    return np.asarray(ys), np.asarray(xs)


YS, XS = _scan_index_arrays()


def _chunk_sizes(k):
    nch = (k + CELLS_PER_CHUNK - 1) // CELLS_PER_CHUNK
    lo = k // nch
    rem = k - lo * nch
    return [lo + 1] * rem + [lo] * (nch - rem)


def make_pt(xc):
    """(B, 32, 32) core batch slice -> (9, 2, TOT_CELLS*B) fp8 patch matrix.

    Half 0 = patch rows 0..8; half 1 = rows 9..15, the ones (bias) row, and a
    zero pad row.  DoubleRow sums W0^T@P0 + W1^T@P1 = full 17-row projection.
    """
    from numpy.lib.stride_tricks import sliding_window_view
    w = sliding_window_view(xc, (GRID, GRID), axis=(1, 2))   # (B, 29, 29, 4, 4)
    p = w[:, YS, XS].reshape(xc.shape[0], TOT_CELLS, GRID * GRID)  # (B, T, 16)
    p = np.ascontiguousarray(p.transpose(2, 1, 0)).reshape(GRID * GRID, -1)
    cols = p.shape[1]
    pt = np.zeros((KH, 2, cols), np.float32)
    pt[:, 0] = p[0:KH]
    pt[0:GRID * GRID - KH, 1] = p[KH:]
    pt[GRID * GRID - KH, 1] = 1.0          # bias row
    return np.ascontiguousarray(pt.astype(F8))


def make_weight_maps(Wx, Uh, Uh2, b, W_out, b_out):
    Wx, Uh, Uh2 = (np.asarray(t, np.float32) for t in (Wx, Uh, Uh2))
    b, W_out, b_out = (np.asarray(t, np.float32) for t in (b, W_out, b_out))
    # recurrence weights as DoubleRow lhsT: [a, 128, kc(2), 3H]
    uh = np.empty((4, 128, 2, H3), np.float32)
    uh2 = np.empty((4, 128, 2, H3), np.float32)
    for a in range(4):
        for kc in range(2):
            uh[a, :, kc, :] = Uh[a][kc * 128:(kc + 1) * 128]
            uh2[a, :, kc, :] = Uh2[a][kc * 128:(kc + 1) * 128]
    # input projection as DoubleRow lhsT: [a, 9, half(2), 3H]
    wx = np.zeros((4, KH, 2, H3), np.float32)
    for a in range(4):
        wx[a, :, 0, :] = Wx[a][0:KH]
        wx[a, 0:GRID * GRID - KH, 1, :] = Wx[a][KH:]
        wx[a, GRID * GRID - KH, 1, :] = b[a]
    wo = np.ascontiguousarray(W_out.reshape(8, 128, OUT_DIM))
    bo = np.ascontiguousarray(b_out.reshape(1, OUT_DIM))
    return {
        "uh": np.ascontiguousarray(uh.astype(F8)),
        "uh2": np.ascontiguousarray(uh2.astype(F8)),
        "wx": np.ascontiguousarray(wx.astype(F8)),
        "wo": wo,
        "bo": bo,
    }


def _build_nc():
    import concourse.bacc as bacc
    import concourse.mybir as mybir
    import concourse.tile as tile

    f32 = mybir.dt.float32
    f8 = mybir.dt.float8e4
    bf16 = mybir.dt.bfloat16
    AF = mybir.ActivationFunctionType
    ALU = mybir.AluOpType
    DR = mybir.MatmulPerfMode.DoubleRow
    CPB = CELLS_PER_CHUNK * B       # 256

    nc = bacc.Bacc("TRN2", target_bir_lowering=False, debug=False,
                   num_devices=N_CORES)
    pt_d = nc.dram_tensor("pt", [KH, 2, TOT_CELLS * B], f8, kind="ExternalInput")
    uh_d = nc.dram_tensor("uh", [4, 128, 2, H3], f8, kind="ExternalInput")
    uh2_d = nc.dram_tensor("uh2", [4, 128, 2, H3], f8, kind="ExternalInput")
    wx_d = nc.dram_tensor("wx", [4, KH, 2, H3], f8, kind="ExternalInput")
    wo_d = nc.dram_tensor("wo", [8, 128, OUT_DIM], f32, kind="ExternalInput")
    bo_d = nc.dram_tensor("bo", [1, OUT_DIM], f32, kind="ExternalInput")
    out_d = nc.dram_tensor("out", [B, OUT_DIM], f32, kind="ExternalOutput")

    with tile.TileContext(nc) as tc:
        from contextlib import ExitStack
        with ExitStack() as ctx:
            const = ctx.enter_context(tc.tile_pool(name="const", bufs=1))
            ptp = ctx.enter_context(tc.tile_pool(name="ptp", bufs=6))
            ps = ctx.enter_context(tc.tile_pool(name="ps", bufs=2, space="PSUM"))
            hps = [ctx.enter_context(tc.tile_pool(name=f"h{a}", bufs=3))
                   for a in range(4)]
            ew = ctx.enter_context(tc.tile_pool(name="ew", bufs=4))
            hd = ctx.enter_context(tc.tile_pool(name="hd", bufs=1))

            # --- resident weights ---
            uh_sb, uh2_sb, wx_sb = {}, {}, {}
            for a in range(4):
                t = const.tile([128, 2, H3], f8, tag=f"uh{a}")
                nc.sync.dma_start(out=t, in_=uh_d[a])
                uh_sb[a] = t
                t = const.tile([128, 2, H3], f8, tag=f"uh2{a}")
                nc.sync.dma_start(out=t, in_=uh2_d[a])
                uh2_sb[a] = t
                t = const.tile([KH, 2, H3], f8, tag=f"wx{a}")
                nc.sync.dma_start(out=t, in_=wx_d[a])
                wx_sb[a] = t
            wo_sb = const.tile([128, 8 * OUT_DIM], f32, tag="wo")
            for c in range(8):
                nc.sync.dma_start(out=wo_sb[:, c * OUT_DIM:(c + 1) * OUT_DIM],
                                  in_=wo_d[c])
            bo_sb = const.tile([1, OUT_DIM], f32, tag="bo")
            nc.sync.dma_start(out=bo_sb, in_=bo_d[:, :])
            ones_sb = const.tile([1, B], f32, tag="ones")
            nc.vector.memset(ones_sb, 1.0)
            zero_h8 = const.tile([128, 2, 2 * B], f8, tag="zeroh8")
            nc.vector.memset(zero_h8, 0.0)
            zero_hb = const.tile([128, 2, 2 * B], bf16, tag="zerohb")
            nc.vector.memset(zero_hb, 0.0)

            # Greedy DVE/Pool balancing: running busy-ns estimate per engine.
            ew_load = {"v": 0.0, "p": 0.0}

            def vop(kind, out, *ins, free, force=None):
                """kind: tt_mul | tt_add | stt_half_sub; free: elem count.
                Picks DVE ('v') or Pool ('p') by projected cost, emits op."""
                psum_in = any(str(t.space).endswith("PSUM") for t in ins)
                b1 = all(mybir.dt.size(t.dtype) == 2 for t in (out, *ins))
                if kind == "stt_half_sub":
                    cv = free * 1.042 + 60
                    cp = 1e9          # TensorScalarPtr unsupported on Pool
                    force = force or "v"
                else:
                    cv = free * (0.521 if (b1 and not psum_in) else 1.042) + 60
                    cp = free * 0.833 / 0.42 + 131
                eng = force or ("v" if ew_load["v"] + cv <= ew_load["p"] + cp
                                else "p")
                e_ = nc.vector if eng == "v" else nc.gpsimd
                ew_load[eng] += cv if eng == "v" else cp
                if kind == "tt_mul":
                    e_.tensor_mul(out, *ins)
                elif kind == "tt_add":
                    e_.tensor_add(out, *ins)
                else:
                    e_.scalar_tensor_tensor(out, ins[0], 0.5, ins[1],
                                            ALU.mult, ALU.subtract)

            def emit_chunk(a, prev8, prevb, s_a, cbase, c0, c1, ht8, htb):
                """prev8/prevb: previous diagonal h tiles [128, 2, (kp+2)B].
                Writes h' of cells c0..c1 into ht8/htb at col (1+c0)B."""
                fd = (c1 - c0) * B
                fe = 2 * fd      # elems per partition for a [2, fd] op
                ptt = ptp.tile([KH, 2, CPB], f8, tag="pt")
                nc.sync.dma_start(
                    out=ptt[:, :, :fd],
                    in_=pt_d[:, :, (cbase + c0) * B:(cbase + c1) * B])
                above = prev8[:, :, (s_a + c0) * B:(s_a + c0) * B + fd]
                left = prev8[:, :, (s_a + 1 + c0) * B:(s_a + 1 + c0) * B + fd]
                above_b = prevb[:, :, (s_a + c0) * B:(s_a + c0) * B + fd]
                left_b = prevb[:, :, (s_a + 1 + c0) * B:(s_a + 1 + c0) * B + fd]

                # P_rz bank kc = [r_kc | z_kc]; P_nx bank kc = [nh_kc | xn_kc]
                P_rz = ps.tile([128, 2, 2, CPB], f32, tag="prz")
                P_nx = ps.tile([128, 2, 2, CPB], f32, tag="pnx")
                for kc in (0, 1):
                    for mc, pst, g, has_in, has_rec in (
                            (kc, P_rz, 0, True, True),
                            (2 + kc, P_rz, 1, True, True),
                            (4 + kc, P_nx, 0, False, True),
                            (4 + kc, P_nx, 1, True, False)):
                        po = pst[:, kc, g, :fd]
                        csl = slice(mc * 128, (mc + 1) * 128)
                        first = True
                        if has_in:
                            nc.tensor.matmul(po, wx_sb[a][:, :, csl],
                                             ptt[:, :, :fd], perf_mode=DR,
                                             start=True, stop=not has_rec)
                            first = False
                        if has_rec:
                            nc.tensor.matmul(po, uh_sb[a][:, :, csl], above,
                                             perf_mode=DR, start=first,
                                             stop=False)
                            nc.tensor.matmul(po, uh2_sb[a][:, :, csl], left,
                                             perf_mode=DR, start=False,
                                             stop=True)

                rzt = ew.tile([128, 2, 2, CPB], bf16, tag="rz")
                nc.scalar.activation(rzt[:, :, :, :fd], P_rz[:, :, :, :fd],
                                     AF.Sigmoid)
                rt = rzt[:, :, 0, :fd]
                zt = rzt[:, :, 1, :fd]
                t1 = ew.tile([128, 2, CPB], bf16, tag="t1")
                vop("tt_mul", t1[:, :, :fd], rt, P_nx[:, :, 0, :fd],
                    free=fe, force="v")
                t2 = ew.tile([128, 2, CPB], bf16, tag="t2")
                vop("tt_add", t2[:, :, :fd], t1[:, :, :fd],
                    P_nx[:, :, 1, :fd], free=fe, force="v")
                nt = ew.tile([128, 2, CPB], bf16, tag="n")
                nc.scalar.activation(nt[:, :, :fd], t2[:, :, :fd], AF.Tanh)
                st = ew.tile([128, 2, CPB], bf16, tag="s")
                vop("tt_add", st[:, :, :fd], above_b, left_b, free=fe)
                dt_ = ew.tile([128, 2, CPB], bf16, tag="d")
                vop("stt_half_sub", dt_[:, :, :fd], st[:, :, :fd],
                    nt[:, :, :fd], free=fe)
                et = ew.tile([128, 2, CPB], bf16, tag="e")
                vop("tt_mul", et[:, :, :fd], zt, dt_[:, :, :fd], free=fe)
                ob = htb[:, :, (1 + c0) * B:(1 + c0) * B + fd]
                vop("tt_add", ob, et[:, :, :fd], nt[:, :, :fd], free=fe)
                o8 = ht8[:, :, (1 + c0) * B:(1 + c0) * B + fd]
                vop("tt_add", o8, et[:, :, :fd], nt[:, :, :fd], free=fe)

            # --- main wavefront, 4 directions interleaved per diagonal ---
            max_nd = max(len(di) for di in DIAG_INFOS)
            for _rep in range(REPEAT):
              h_prev = {a: None for a in range(4)}
              for d in range(max_nd):
                 for a in range(4):
                    if d >= len(DIAG_INFOS[a]):
                        continue
                    ilo, ihi, cbase = DIAG_INFOS[a][d]
                    k = ihi - ilo + 1
                    ht8 = hps[a].tile([128, 2, (k + 2) * B], f8, tag=f"h{a}")
                    htb = hps[a].tile([128, 2, (k + 2) * B], bf16, tag=f"hb{a}")
                    for t in (ht8, htb):
                        nc.vector.memset(t[:, :, 0:B], 0.0)
                        nc.gpsimd.memset(t[:, :, (k + 1) * B:(k + 2) * B], 0.0)
                    if d == 0:
                        prev8, prevb, k_prev, ilo_prev = zero_h8, zero_hb, 0, 0
                    else:
                        prev8, prevb, k_prev, ilo_prev = h_prev[a]
                    s_a = ilo - ilo_prev
                    assert 0 <= s_a and s_a + k <= k_prev + 2, (a, d)
                    c0 = 0
                    for cs in _chunk_sizes(k):
                        emit_chunk(a, prev8, prevb, s_a, cbase, c0, c0 + cs,
                                   ht8, htb)
                        c0 += cs
                    h_prev[a] = (ht8, htb, k, ilo)

            # --- head: logits = hcat @ W_out + b_out ; log_softmax ---
            hfin = []
            for a in range(4):
                _, htb, k, _ = h_prev[a]
                assert k == 1
                for kc in (0, 1):
                    t = hd.tile([128, B], f32, tag=f"hf{a}{kc}")
                    nc.scalar.copy(t, htb[:, kc, B:2 * B])
                    hfin.append(t)
            pl_t = ps.tile([128, 2, 2, CPB], f32, tag="prz")
            pl = pl_t[:B, 0, 0, :OUT_DIM]
            for c, t in enumerate(hfin):
                nc.tensor.matmul(pl, t, wo_sb[:, c * OUT_DIM:(c + 1) * OUT_DIM],
                                 start=(c == 0), stop=False)
            nc.tensor.matmul(pl, ones_sb[:1, :B], bo_sb, start=False, stop=True)
            mx = hd.tile([B, 1], f32, tag="mx")
            nc.vector.reduce_max(mx, pl, axis=mybir.AxisListType.X)
            nmx = hd.tile([B, 1], f32, tag="nmx")
            nc.vector.tensor_scalar_mul(nmx, mx, -1.0)
            exv = hd.tile([B, OUT_DIM], f32, tag="exv")
            nc.scalar.activation(exv, pl, AF.Exp, bias=nmx, scale=1.0)
            sm = hd.tile([B, 1], f32, tag="sm")
            nc.vector.reduce_sum(sm, exv, axis=mybir.AxisListType.X)
            lnz = hd.tile([B, 1], f32, tag="lnz")
            nc.scalar.activation(lnz, sm, AF.Ln)
            tot = hd.tile([B, 1], f32, tag="tot")
            nc.vector.tensor_add(tot, lnz, mx)
            ntot = hd.tile([B, 1], f32, tag="ntot")
            nc.vector.tensor_scalar_mul(ntot, tot, -1.0)
            ot = hd.tile([B, OUT_DIM], f32, tag="ot")
            nc.scalar.activation(ot, pl, AF.Identity, bias=ntot, scale=1.0)
            nc.sync.dma_start(out=out_d[:, :], in_=ot)

    nc.compile()
    return nc


_CACHE = {}


def get_nc():
    if "nc" not in _CACHE:
        _CACHE["nc"] = _build_nc()
    return _CACHE["nc"]


def make_in_maps(x, Wx, Uh, Uh2, b, W_out, b_out):
    x = np.asarray(x, np.float32)
    wm = make_weight_maps(Wx, Uh, Uh2, b, W_out, b_out)
    in_maps = []
    for c in range(N_CORES):
        xc = x[c * B:(c + 1) * B]
        m = dict(wm)
        m["pt"] = make_pt(xc)
        in_maps.append(m)
    return in_maps


def kernel(x, Wx, Uh, Uh2, b, W_out, b_out):
    from concourse.bass_utils import run_bass_kernel_spmd
    nc = get_nc()
    in_maps = make_in_maps(x, Wx, Uh, Uh2, b, W_out, b_out)
    res = run_bass_kernel_spmd(nc, in_maps, list(range(N_CORES)))
    out = np.concatenate([res.results[c]["out"] for c in range(N_CORES)], axis=0)
    return out.astype(np.float32)


# revision 20
# speedup vs baseline: 1.0612x; 1.0612x over previous
"""MD-RNN (4-direction 2D GRU) Trainium2 kernel, fp8 DoubleRow edition.

Sharding: 8-way data-parallel over batch (B=256 -> 32 per core); each core runs
all 4 directional 2D-GRU scans as anti-diagonal wavefronts, interleaved so all
engines stay busy.

Layout (hidden-on-partition, "transposed"):
  - hidden states h^T: (128 partitions = hidden chunk kc in {0,1}, cells*B free)
    stored twice: fp8e4m3 (matmul operand) and bf16 (skip-path + head).
  - all matmuls are fp8 DoubleRow (2 stacked K=128 k-tiles per instruction at
    0.5 cycles/row): per 8-cell chunk, 18 DR instructions produce psum tiles
    P_r, P_z (input+bias+recurrence), P_nh (recurrence only), P_xn (input).
  - gate math: sigmoid/tanh on Act; t1=r*nh, t2=t1+xn, d=0.5*s-n, e=z*d,
    h'Bf=n+e on DVE; s=h+h2 and h'F8=n+e on Pool.

The patch tensor (im2col of x + bias ones row, split into two 9-row halves for
DoubleRow) is built host-side and streamed per-chunk from DRAM.
"""

import numpy as np
import ml_dtypes

GRID = 4
N_IMG = 32
S = N_IMG - (GRID - 1)          # 29 patch positions per axis
B_FULL = 256
N_CORES = 8
B = B_FULL // N_CORES           # 32 batch per core
H = 256
H3 = 3 * H                      # 768
OUT_DIM = 10
K_IN = GRID * GRID + 1          # 16 patch elems + ones row (bias trick)
KH = 9                          # DoubleRow half-height of the input projection

FWD = list(range(S))                 # 29 entries
BWD = list(range(S - 2, -1, -1))     # 28 entries (reference off-by-one kept)
DIRS = [(FWD, FWD), (BWD, FWD), (FWD, BWD), (BWD, BWD)]

CELLS_PER_CHUNK = 16            # 16 cells * B=32 = 512 cols; 8 psum banks/chunk
REPEAT = 1                      # body repetitions (timing calibration only)
SKIP_MM = False                 # timing experiment: drop matmuls
SKIP_EW = False                 # timing experiment: drop elementwise gate math

F8 = ml_dtypes.float8_e4m3
BF = ml_dtypes.bfloat16


def _diag_infos():
    """Per direction: list over diagonals of (i_lo, i_hi, global cell base)."""
    infos = []
    base = 0
    for (yi, xi) in DIRS:
        ny, nx = len(yi), len(xi)
        diags = []
        for d in range(ny + nx - 1):
            ilo = max(0, d - (nx - 1))
            ihi = min(d, ny - 1)
            diags.append((ilo, ihi, base))
            base += ihi - ilo + 1
        infos.append(diags)
    return infos, base


DIAG_INFOS, TOT_CELLS = _diag_infos()


def _scan_index_arrays():
    """Image-space (y, x) of every cell in pt order (dir-major, diag-major)."""
    ys, xs = [], []
    for a, (yi, xi) in enumerate(DIRS):
        for d, (ilo, ihi, _) in enumerate(DIAG_INFOS[a]):
            for i in range(ilo, ihi + 1):
                ys.append(yi[i])
                xs.append(xi[d - i])
    return np.asarray(ys), np.asarray(xs)


YS, XS = _scan_index_arrays()


def _chunk_sizes(k):
    nch = (k + CELLS_PER_CHUNK - 1) // CELLS_PER_CHUNK
    lo = k // nch
    rem = k - lo * nch
    return [lo + 1] * rem + [lo] * (nch - rem)


def make_pt(xc):
    """(B, 32, 32) core batch slice -> (9, 2, TOT_CELLS*B) fp8 patch matrix.

    Half 0 = patch rows 0..8; half 1 = rows 9..15, the ones (bias) row, and a
    zero pad row.  DoubleRow sums W0^T@P0 + W1^T@P1 = full 17-row projection.
    """
    from numpy.lib.stride_tricks import sliding_window_view
    w = sliding_window_view(xc, (GRID, GRID), axis=(1, 2))   # (B, 29, 29, 4, 4)
    p = w[:, YS, XS].reshape(xc.shape[0], TOT_CELLS, GRID * GRID)  # (B, T, 16)
    p = np.ascontiguousarray(p.transpose(2, 1, 0)).reshape(GRID * GRID, -1)
    cols = p.shape[1]
    pt = np.zeros((KH, 2, cols), np.float32)
    pt[:, 0] = p[0:KH]
    pt[0:GRID * GRID - KH, 1] = p[KH:]
    pt[GRID * GRID - KH, 1] = 1.0          # bias row
    return np.ascontiguousarray(pt.astype(F8))


def make_weight_maps(Wx, Uh, Uh2, b, W_out, b_out):
    Wx, Uh, Uh2 = (np.asarray(t, np.float32) for t in (Wx, Uh, Uh2))
    b, W_out, b_out = (np.asarray(t, np.float32) for t in (b, W_out, b_out))
    # recurrence weights as DoubleRow lhsT: [a, 128, kc(2), 3H]
    uh = np.empty((4, 128, 2, H3), np.float32)
    uh2 = np.empty((4, 128, 2, H3), np.float32)
    for a in range(4):
        for kc in range(2):
            uh[a, :, kc, :] = Uh[a][kc * 128:(kc + 1) * 128]
            uh2[a, :, kc, :] = Uh2[a][kc * 128:(kc + 1) * 128]
    # input projection as DoubleRow lhsT: [a, 9, half(2), 3H]
    wx = np.zeros((4, KH, 2, H3), np.float32)
    for a in range(4):
        wx[a, :, 0, :] = Wx[a][0:KH]
        wx[a, 0:GRID * GRID - KH, 1, :] = Wx[a][KH:]
        wx[a, GRID * GRID - KH, 1, :] = b[a]
    wo = np.ascontiguousarray(2.0 * W_out.reshape(8, 128, OUT_DIM))
    bo = np.ascontiguousarray(b_out.reshape(1, OUT_DIM))
    return {
        "uh": np.ascontiguousarray(uh.astype(F8)),
        "uh2": np.ascontiguousarray(uh2.astype(F8)),
        "wx": np.ascontiguousarray(wx.astype(F8)),
        "wo": wo,
        "bo": bo,
    }


def _build_nc():
    import concourse.bacc as bacc
    import concourse.mybir as mybir
    import concourse.tile as tile

    f32 = mybir.dt.float32
    f8 = mybir.dt.float8e4
    bf16 = mybir.dt.bfloat16
    AF = mybir.ActivationFunctionType
    ALU = mybir.AluOpType
    DR = mybir.MatmulPerfMode.DoubleRow
    CPB = CELLS_PER_CHUNK * B

    PS_BUFS = 1
    nc = bacc.Bacc("TRN2", target_bir_lowering=False, debug=False,
                   num_devices=N_CORES)
    pt_d = nc.dram_tensor("pt", [KH, 2, TOT_CELLS * B], f8, kind="ExternalInput")
    uh_d = nc.dram_tensor("uh", [4, 128, 2, H3], f8, kind="ExternalInput")
    uh2_d = nc.dram_tensor("uh2", [4, 128, 2, H3], f8, kind="ExternalInput")
    wx_d = nc.dram_tensor("wx", [4, KH, 2, H3], f8, kind="ExternalInput")
    wo_d = nc.dram_tensor("wo", [8, 128, OUT_DIM], f32, kind="ExternalInput")
    bo_d = nc.dram_tensor("bo", [1, OUT_DIM], f32, kind="ExternalInput")
    out_d = nc.dram_tensor("out", [B, OUT_DIM], f32, kind="ExternalOutput")

    with tile.TileContext(nc) as tc:
        from contextlib import ExitStack
        with ExitStack() as ctx:
            const = ctx.enter_context(tc.tile_pool(name="const", bufs=1))
            ptp = ctx.enter_context(tc.tile_pool(name="ptp", bufs=6))
            ps = ctx.enter_context(tc.tile_pool(name="ps", bufs=PS_BUFS, space="PSUM"))
            hps = [ctx.enter_context(tc.tile_pool(name=f"h{a}", bufs=3))
                   for a in range(4)]
            ew = ctx.enter_context(tc.tile_pool(name="ew", bufs=4))
            hd = ctx.enter_context(tc.tile_pool(name="hd", bufs=1))

            # --- resident weights ---
            uh_sb, uh2_sb, wx_sb = {}, {}, {}
            for a in range(4):
                t = const.tile([128, 2, H3], f8, tag=f"uh{a}")
                nc.sync.dma_start(out=t, in_=uh_d[a])
                uh_sb[a] = t
                t = const.tile([128, 2, H3], f8, tag=f"uh2{a}")
                nc.sync.dma_start(out=t, in_=uh2_d[a])
                uh2_sb[a] = t
                t = const.tile([KH, 2, H3], f8, tag=f"wx{a}")
                nc.sync.dma_start(out=t, in_=wx_d[a])
                wx_sb[a] = t
            wo_sb = const.tile([128, 8 * OUT_DIM], f32, tag="wo")
            for c in range(8):
                nc.sync.dma_start(out=wo_sb[:, c * OUT_DIM:(c + 1) * OUT_DIM],
                                  in_=wo_d[c])
            bo_sb = const.tile([1, OUT_DIM], f32, tag="bo")
            nc.sync.dma_start(out=bo_sb, in_=bo_d[:, :])
            ones_sb = const.tile([1, B], f32, tag="ones")
            nc.vector.memset(ones_sb, 1.0)
            zero_h8 = const.tile([128, 2, 2 * B], f8, tag="zeroh8")
            nc.vector.memset(zero_h8, 0.0)
            zero_hb = const.tile([128, 2, 2 * B], bf16, tag="zerohb")
            nc.vector.memset(zero_hb, 0.0)

            # Greedy DVE/Pool balancing: running busy-ns estimate per engine.
            ew_load = {"v": 0.0, "p": 0.0}

            def vop(kind, out, *ins, free, force=None, scalar=None):
                """kind: tt_mul | tt_add | tt_sub | tsp_mul; free: per-lane
                elem count. Picks DVE ('v') or Pool ('p') by projected cost."""
                psum_in = any(str(t.space).endswith("PSUM") for t in ins)
                b1 = all(mybir.dt.size(t.dtype) == 2 for t in (out, *ins))
                if kind == "tsp_mul":
                    cv = free * (0.26 if b1 and not psum_in else 1.042) + 60
                    cp = 1e9          # TensorScalarPtr unsupported on Pool
                    force = force or "v"
                else:
                    cv = (free * (0.521 if (b1 and not psum_in) else 1.042)
                          + (125 if psum_in else 60))
                    cp = free * 1.984 + 95
                    if psum_in:
                        cp = 1e9      # Pool+PSUM untested on walrus
                eng = force or ("v" if ew_load["v"] + cv <= ew_load["p"] + cp
                                else "p")
                e_ = nc.vector if eng == "v" else nc.gpsimd
                ew_load[eng] += cv if eng == "v" else cp
                if kind == "tt_mul":
                    e_.tensor_mul(out, *ins)
                elif kind == "tt_add":
                    e_.tensor_add(out, *ins)
                elif kind == "tt_sub":
                    e_.tensor_tensor(out, ins[0], ins[1], mybir.AluOpType.subtract)
                else:
                    e_.tensor_scalar_mul(out, ins[0], scalar)

            def emit_chunk(a, prev8, prevb, s_a, cbase, c0, c1, ht8, htb):
                """One 16-cell chunk: psum prz+pnx (4 banks each, bufs=1).
                Matmuls kc-grouped so sigma/t1/t2 free psum halves early."""
                fd = (c1 - c0) * B
                ptt = ptp.tile([KH, 2, CPB], f8, tag="pt")
                nc.sync.dma_start(
                    out=ptt[:, :, :fd],
                    in_=pt_d[:, :, (cbase + c0) * B:(cbase + c1) * B])
                above = prev8[:, :, (s_a + c0) * B:(s_a + c0) * B + fd]
                left = prev8[:, :, (s_a + 1 + c0) * B:(s_a + 1 + c0) * B + fd]
                above_b = prevb[:, :, (s_a + c0) * B:(s_a + c0) * B + fd]
                left_b = prevb[:, :, (s_a + 1 + c0) * B:(s_a + 1 + c0) * B + fd]

                # P_rz[kc] banks: [r_kc | z_kc]; P_nx[kc]: [nh_kc | xn_kc]
                P_rz = ps.tile([128, 2, 2, CPB], f32, tag="prz")
                P_nx = ps.tile([128, 2, 2, CPB], f32, tag="pnx")
                if not SKIP_MM:
                    for kc in (0, 1):     # r,z gates for hidden chunk kc
                        for mc, g in ((kc, 0), (2 + kc, 1)):
                            csl = slice(mc * 128, (mc + 1) * 128)
                            po = P_rz[:, kc, g, :fd]
                            nc.tensor.matmul(po, wx_sb[a][:, :, csl],
                                             ptt[:, :, :fd], perf_mode=DR,
                                             start=True, stop=False)
                            nc.tensor.matmul(po, uh_sb[a][:, :, csl], above,
                                             perf_mode=DR, start=False,
                                             stop=False)
                            nc.tensor.matmul(po, uh2_sb[a][:, :, csl], left,
                                             perf_mode=DR, start=False,
                                             stop=True)
                    for kc in (0, 1):     # nh (rec only) + xn (input only)
                        mc = 4 + kc
                        csl = slice(mc * 128, (mc + 1) * 128)
                        po = P_nx[:, kc, 0, :fd]
                        nc.tensor.matmul(po, uh_sb[a][:, :, csl], above,
                                         perf_mode=DR, start=True, stop=False)
                        nc.tensor.matmul(po, uh2_sb[a][:, :, csl], left,
                                         perf_mode=DR, start=False, stop=True)
                        nc.tensor.matmul(P_nx[:, kc, 1, :fd],
                                         wx_sb[a][:, :, csl], ptt[:, :, :fd],
                                         perf_mode=DR, start=True, stop=True)

                if SKIP_EW:
                    ob = htb[:, :, (1 + c0) * B:(1 + c0) * B + fd]
                    nc.vector.memset(ob, 0.25)
                    o8 = ht8[:, :, (1 + c0) * B:(1 + c0) * B + fd]
                    nc.gpsimd.memset(o8, 0.25)
                    return
                fe = 2 * fd
                # kc-split psum consumers (free the 4-bank tiles half at a
                # time so the next chunk's matmuls can start early)
                rzt = ew.tile([128, 2, 2, CPB], bf16, tag="rz")
                t1 = ew.tile([128, 2, CPB], bf16, tag="t1")
                t2 = ew.tile([128, 2, CPB], bf16, tag="t2")
                for kc in (0, 1):
                    nc.scalar.activation(rzt[:, kc, :, :fd],
                                         P_rz[:, kc, :, :fd], AF.Sigmoid)
                vop("tt_mul", t1[:, :, :fd], rzt[:, :, 0, :fd],
                    P_nx[:, :, 0, :fd], free=fe, force="v")
                vop("tt_add", t2[:, :, :fd], t1[:, :, :fd],
                    P_nx[:, :, 1, :fd], free=fe, force="v")
                # merged remainder
                nt = ew.tile([128, 2, CPB], bf16, tag="n")
                nc.scalar.activation(nt[:, :, :fd], t2[:, :, :fd], AF.Tanh)
                st = ew.tile([128, 2, CPB], bf16, tag="s")
                vop("tt_add", st[:, :, :fd], above_b, left_b, free=fe)
                dt_ = ew.tile([128, 2, CPB], bf16, tag="d")
                vop("tt_sub", dt_[:, :, :fd], st[:, :, :fd], nt[:, :, :fd],
                    free=fe)
                et = ew.tile([128, 2, CPB], bf16, tag="e")
                vop("tt_mul", et[:, :, :fd], rzt[:, :, 1, :fd], dt_[:, :, :fd],
                    free=fe)
                tmp = ew.tile([128, 2, CPB], bf16, tag="hsum")
                vop("tt_add", tmp[:, :, :fd], et[:, :, :fd], nt[:, :, :fd],
                    free=fe)
                o8 = ht8[:, :, (1 + c0) * B:(1 + c0) * B + fd]
                vop("tsp_mul", o8, tmp[:, :, :fd], free=fe, scalar=1.0)
                ob = htb[:, :, (1 + c0) * B:(1 + c0) * B + fd]
                vop("tsp_mul", ob, tmp[:, :, :fd], free=fe, scalar=0.5)

            # --- main wavefront, 4 directions interleaved per diagonal ---
            max_nd = max(len(di) for di in DIAG_INFOS)
            for _rep in range(REPEAT):
              h_prev = {a: None for a in range(4)}
              for d in range(max_nd):
                 for a in range(4):
                    if d >= len(DIAG_INFOS[a]):
                        continue
                    ilo, ihi, cbase = DIAG_INFOS[a][d]
                    k = ihi - ilo + 1
                    ht8 = hps[a].tile([128, 2, (k + 2) * B], f8, tag=f"h{a}")
                    htb = hps[a].tile([128, 2, (k + 2) * B], bf16, tag=f"hb{a}")
                    for t in (ht8, htb):
                        nc.vector.memset(t[:, :, 0:B], 0.0)
                        nc.gpsimd.memset(t[:, :, (k + 1) * B:(k + 2) * B], 0.0)
                    if d == 0:
                        prev8, prevb, k_prev, ilo_prev = zero_h8, zero_hb, 0, 0
                    else:
                        prev8, prevb, k_prev, ilo_prev = h_prev[a]
                    s_a = ilo - ilo_prev
                    assert 0 <= s_a and s_a + k <= k_prev + 2, (a, d)
                    c0 = 0
                    for cs in _chunk_sizes(k):
                        emit_chunk(a, prev8, prevb, s_a, cbase, c0, c0 + cs,
                                   ht8, htb)
                        c0 += cs
                    h_prev[a] = (ht8, htb, k, ilo)

            # --- head: logits = hcat @ W_out + b_out ; log_softmax ---
            hfin = []
            for a in range(4):
                _, htb, k, _ = h_prev[a]
                assert k == 1
                for kc in (0, 1):
                    t = hd.tile([128, B], f32, tag=f"hf{a}{kc}")
                    nc.scalar.copy(t, htb[:, kc, B:2 * B])
                    hfin.append(t)
            pl_t = ps.tile([128, 2, 2, CPB], f32, tag="prz")
            pl = pl_t[:B, 0, 0, :OUT_DIM]
            for c, t in enumerate(hfin):
                nc.tensor.matmul(pl, t, wo_sb[:, c * OUT_DIM:(c + 1) * OUT_DIM],
                                 start=(c == 0), stop=False)
            nc.tensor.matmul(pl, ones_sb[:1, :B], bo_sb, start=False, stop=True)
            mx = hd.tile([B, 1], f32, tag="mx")
            nc.vector.reduce_max(mx, pl, axis=mybir.AxisListType.X)
            nmx = hd.tile([B, 1], f32, tag="nmx")
            nc.vector.tensor_scalar_mul(nmx, mx, -1.0)
            exv = hd.tile([B, OUT_DIM], f32, tag="exv")
            nc.scalar.activation(exv, pl, AF.Exp, bias=nmx, scale=1.0)
            sm = hd.tile([B, 1], f32, tag="sm")
            nc.vector.reduce_sum(sm, exv, axis=mybir.AxisListType.X)
            lnz = hd.tile([B, 1], f32, tag="lnz")
            nc.scalar.activation(lnz, sm, AF.Ln)
            tot = hd.tile([B, 1], f32, tag="tot")
            nc.vector.tensor_add(tot, lnz, mx)
            ntot = hd.tile([B, 1], f32, tag="ntot")
            nc.vector.tensor_scalar_mul(ntot, tot, -1.0)
            ot = hd.tile([B, OUT_DIM], f32, tag="ot")
            nc.scalar.activation(ot, pl, AF.Identity, bias=ntot, scale=1.0)
            nc.sync.dma_start(out=out_d[:, :], in_=ot)

    _dedupe_ldweights(nc)
    nc.compile()
    return nc


def _dedupe_ldweights(nc):
    """Drop InstLdweights that reload the stationary weights already loaded
    by the immediately preceding load (weight-major groups emit the same
    lhsT for consecutive matmuls). PE executes in order, so the stationary
    contents are still valid; only clean (no-sync) repeats are dropped."""
    ndrop = 0
    for blk in nc.m.functions[0].blocks:
        last_sig = None
        drops = []
        for idx, inst in enumerate(blk.instructions):
            if not str(inst.engine).endswith("PE"):
                continue
            nm = type(inst).__name__
            if nm == "InstLdweights":
                ap = inst.ins[0]
                sig = (getattr(ap, "memref", None), getattr(ap, "offset", None),
                       str(getattr(ap, "ap", None)), str(inst.perf_mode),
                       str(getattr(inst, "tile_position", None)))
                si = inst.sync_info
                clean = si is None or (len(si.on_wait) == 0 and
                                       len(si.on_update) == 0)
                if sig == last_sig and clean:
                    drops.append(idx)
                else:
                    last_sig = sig
            elif nm in ("InstMatmult", "InstEventSemaphore"):
                pass
            else:
                last_sig = None
        for idx in reversed(drops):
            del blk.instructions[idx]
        ndrop += len(drops)
    return ndrop


_CACHE = {}


def get_nc():
    if "nc" not in _CACHE:
        _CACHE["nc"] = _build_nc()
    return _CACHE["nc"]


def make_in_maps(x, Wx, Uh, Uh2, b, W_out, b_out):
    x = np.asarray(x, np.float32)
    wm = make_weight_maps(Wx, Uh, Uh2, b, W_out, b_out)
    in_maps = []
    for c in range(N_CORES):
        xc = x[c * B:(c + 1) * B]
        m = dict(wm)
        m["pt"] = make_pt(xc)
        in_maps.append(m)
    return in_maps


def kernel(x, Wx, Uh, Uh2, b, W_out, b_out):
    from concourse.bass_utils import run_bass_kernel_spmd
    nc = get_nc()
    in_maps = make_in_maps(x, Wx, Uh, Uh2, b, W_out, b_out)
    res = run_bass_kernel_spmd(nc, in_maps, list(range(N_CORES)))
    out = np.concatenate([res.results[c]["out"] for c in range(N_CORES)], axis=0)
    return out.astype(np.float32)


# revision 21
# speedup vs baseline: 1.1781x; 1.1102x over previous
"""MD-RNN (4-direction 2D GRU) Trainium2 kernel, fp8 DoubleRow edition.

Sharding: 8-way data-parallel over batch (B=256 -> 32 per core); each core runs
all 4 directional 2D-GRU scans as anti-diagonal wavefronts, interleaved so all
engines stay busy.

Layout (hidden-on-partition, "transposed"):
  - hidden states h^T: (128 partitions = hidden chunk kc in {0,1}, cells*B free)
    stored twice: fp8e4m3 (matmul operand) and bf16 (skip-path + head).
  - all matmuls are fp8 DoubleRow (2 stacked K=128 k-tiles per instruction at
    0.5 cycles/row): per 8-cell chunk, 18 DR instructions produce psum tiles
    P_r, P_z (input+bias+recurrence), P_nh (recurrence only), P_xn (input).
  - gate math: sigmoid/tanh on Act; t1=r*nh, t2=t1+xn, d=0.5*s-n, e=z*d,
    h'Bf=n+e on DVE; s=h+h2 and h'F8=n+e on Pool.

The patch tensor (im2col of x + bias ones row, split into two 9-row halves for
DoubleRow) is built host-side and streamed per-chunk from DRAM.
"""

import numpy as np
import ml_dtypes

GRID = 4
N_IMG = 32
S = N_IMG - (GRID - 1)          # 29 patch positions per axis
B_FULL = 256
N_CORES = 8
B = B_FULL // N_CORES           # 32 batch per core
H = 256
H3 = 3 * H                      # 768
OUT_DIM = 10
K_IN = GRID * GRID + 1          # 16 patch elems + ones row (bias trick)
KH = 9                          # DoubleRow half-height of the input projection

FWD = list(range(S))                 # 29 entries
BWD = list(range(S - 2, -1, -1))     # 28 entries (reference off-by-one kept)
DIRS = [(FWD, FWD), (BWD, FWD), (FWD, BWD), (BWD, BWD)]

CELLS_PER_CHUNK = 16            # 16 cells * B=32 = 512 cols; 8 psum banks/chunk
REPEAT = 1                      # body repetitions (timing calibration only)
SKIP_MM = False                 # timing experiment: drop matmuls
SKIP_EW = False                 # timing experiment: drop elementwise gate math

F8 = ml_dtypes.float8_e4m3
BF = ml_dtypes.bfloat16


def _diag_infos():
    """Per direction: list over diagonals of (i_lo, i_hi, global cell base)."""
    infos = []
    base = 0
    for (yi, xi) in DIRS:
        ny, nx = len(yi), len(xi)
        diags = []
        for d in range(ny + nx - 1):
            ilo = max(0, d - (nx - 1))
            ihi = min(d, ny - 1)
            diags.append((ilo, ihi, base))
            base += ihi - ilo + 1
        infos.append(diags)
    return infos, base


DIAG_INFOS, TOT_CELLS = _diag_infos()


def _scan_index_arrays():
    """Image-space (y, x) of every cell in pt order (dir-major, diag-major)."""
    ys, xs = [], []
    for a, (yi, xi) in enumerate(DIRS):
        for d, (ilo, ihi, _) in enumerate(DIAG_INFOS[a]):
            for i in range(ilo, ihi + 1):
                ys.append(yi[i])
                xs.append(xi[d - i])
    return np.asarray(ys), np.asarray(xs)


YS, XS = _scan_index_arrays()


def _chunk_sizes(k):
    nch = (k + CELLS_PER_CHUNK - 1) // CELLS_PER_CHUNK
    lo = k // nch
    rem = k - lo * nch
    return [lo + 1] * rem + [lo] * (nch - rem)


def make_pt(xc):
    """(B, 32, 32) core batch slice -> (9, 2, TOT_CELLS*B) fp8 patch matrix.

    Half 0 = patch rows 0..8; half 1 = rows 9..15, the ones (bias) row, and a
    zero pad row.  DoubleRow sums W0^T@P0 + W1^T@P1 = full 17-row projection.
    """
    from numpy.lib.stride_tricks import sliding_window_view
    w = sliding_window_view(xc, (GRID, GRID), axis=(1, 2))   # (B, 29, 29, 4, 4)
    p = w[:, YS, XS].reshape(xc.shape[0], TOT_CELLS, GRID * GRID)  # (B, T, 16)
    p = np.ascontiguousarray(p.transpose(2, 1, 0)).reshape(GRID * GRID, -1)
    cols = p.shape[1]
    pt = np.zeros((KH, 2, cols), np.float32)
    pt[:, 0] = p[0:KH]
    pt[0:GRID * GRID - KH, 1] = p[KH:]
    pt[GRID * GRID - KH, 1] = 1.0          # bias row
    return np.ascontiguousarray(pt.astype(F8))


def make_weight_maps(Wx, Uh, Uh2, b, W_out, b_out):
    Wx, Uh, Uh2 = (np.asarray(t, np.float32) for t in (Wx, Uh, Uh2))
    b, W_out, b_out = (np.asarray(t, np.float32) for t in (b, W_out, b_out))
    # recurrence weights as DoubleRow lhsT: [a, 128, kc(2), 3H]
    uh = np.empty((4, 128, 2, H3), np.float32)
    uh2 = np.empty((4, 128, 2, H3), np.float32)
    for a in range(4):
        for kc in range(2):
            uh[a, :, kc, :] = Uh[a][kc * 128:(kc + 1) * 128]
            uh2[a, :, kc, :] = Uh2[a][kc * 128:(kc + 1) * 128]
    # input projection as DoubleRow lhsT: [a, 9, half(2), 3H]
    wx = np.zeros((4, KH, 2, H3), np.float32)
    for a in range(4):
        wx[a, :, 0, :] = Wx[a][0:KH]
        wx[a, 0:GRID * GRID - KH, 1, :] = Wx[a][KH:]
        wx[a, GRID * GRID - KH, 1, :] = b[a]
    wo = np.ascontiguousarray(2.0 * W_out.reshape(8, 128, OUT_DIM))
    bo = np.ascontiguousarray(b_out.reshape(1, OUT_DIM))
    return {
        "uh": np.ascontiguousarray(uh.astype(F8)),
        "uh2": np.ascontiguousarray(uh2.astype(F8)),
        "wx": np.ascontiguousarray(wx.astype(F8)),
        "wo": wo,
        "bo": bo,
    }


def _build_nc():
    import concourse.bacc as bacc
    import concourse.mybir as mybir
    import concourse.tile as tile

    f32 = mybir.dt.float32
    f8 = mybir.dt.float8e4
    bf16 = mybir.dt.bfloat16
    AF = mybir.ActivationFunctionType
    ALU = mybir.AluOpType
    DR = mybir.MatmulPerfMode.DoubleRow
    CPB = CELLS_PER_CHUNK * B

    PS_BUFS = 1
    nc = bacc.Bacc("TRN2", target_bir_lowering=False, debug=False,
                   num_devices=N_CORES)
    pt_d = nc.dram_tensor("pt", [KH, 2, TOT_CELLS * B], f8, kind="ExternalInput")
    uh_d = nc.dram_tensor("uh", [4, 128, 2, H3], f8, kind="ExternalInput")
    uh2_d = nc.dram_tensor("uh2", [4, 128, 2, H3], f8, kind="ExternalInput")
    wx_d = nc.dram_tensor("wx", [4, KH, 2, H3], f8, kind="ExternalInput")
    wo_d = nc.dram_tensor("wo", [8, 128, OUT_DIM], f32, kind="ExternalInput")
    bo_d = nc.dram_tensor("bo", [1, OUT_DIM], f32, kind="ExternalInput")
    out_d = nc.dram_tensor("out", [B, OUT_DIM], f32, kind="ExternalOutput")

    with tile.TileContext(nc) as tc:
        from contextlib import ExitStack
        with ExitStack() as ctx:
            const = ctx.enter_context(tc.tile_pool(name="const", bufs=1))
            ptp = ctx.enter_context(tc.tile_pool(name="ptp", bufs=8))
            ps = ctx.enter_context(tc.tile_pool(name="ps", bufs=PS_BUFS, space="PSUM"))
            hps = [ctx.enter_context(tc.tile_pool(name=f"h{a}", bufs=3))
                   for a in range(4)]
            ew = ctx.enter_context(tc.tile_pool(name="ew", bufs=6))
            hd = ctx.enter_context(tc.tile_pool(name="hd", bufs=1))

            # --- resident weights ---
            uh_sb, uh2_sb, wx_sb = {}, {}, {}
            for a in range(4):
                t = const.tile([128, 2, H3], f8, tag=f"uh{a}")
                nc.sync.dma_start(out=t, in_=uh_d[a])
                uh_sb[a] = t
                t = const.tile([128, 2, H3], f8, tag=f"uh2{a}")
                nc.sync.dma_start(out=t, in_=uh2_d[a])
                uh2_sb[a] = t
                t = const.tile([KH, 2, H3], f8, tag=f"wx{a}")
                nc.sync.dma_start(out=t, in_=wx_d[a])
                wx_sb[a] = t
            wo_sb = const.tile([128, 8 * OUT_DIM], f32, tag="wo")
            for c in range(8):
                nc.sync.dma_start(out=wo_sb[:, c * OUT_DIM:(c + 1) * OUT_DIM],
                                  in_=wo_d[c])
            bo_sb = const.tile([1, OUT_DIM], f32, tag="bo")
            nc.sync.dma_start(out=bo_sb, in_=bo_d[:, :])
            ones_sb = const.tile([1, B], f32, tag="ones")
            nc.vector.memset(ones_sb, 1.0)
            zero_h8 = const.tile([128, 2, 2 * B], f8, tag="zeroh8")
            nc.vector.memset(zero_h8, 0.0)
            zero_hb = const.tile([128, 2, 2 * B], bf16, tag="zerohb")
            nc.vector.memset(zero_hb, 0.0)

            # Greedy DVE/Pool balancing: running busy-ns estimate per engine.
            ew_load = {"v": 0.0, "p": 0.0}

            def vop(kind, out, *ins, free, force=None, scalar=None):
                """kind: tt_mul | tt_add | tt_sub | tsp_mul; free: per-lane
                elem count. Picks DVE ('v') or Pool ('p') by projected cost."""
                psum_in = any(str(t.space).endswith("PSUM") for t in ins)
                b1 = all(mybir.dt.size(t.dtype) == 2 for t in (out, *ins))
                if kind == "tsp_mul":
                    cv = free * (0.26 if b1 and not psum_in else 1.042) + 60
                    cp = 1e9          # TensorScalarPtr unsupported on Pool
                    force = force or "v"
                else:
                    cv = (free * (0.521 if (b1 and not psum_in) else 1.042)
                          + (125 if psum_in else 60))
                    cp = free * 1.984 + 95
                    if psum_in:
                        cp = 1e9      # Pool+PSUM untested on walrus
                eng = force or ("v" if ew_load["v"] + cv <= ew_load["p"] + cp
                                else "p")
                e_ = nc.vector if eng == "v" else nc.gpsimd
                ew_load[eng] += cv if eng == "v" else cp
                if kind == "tt_mul":
                    e_.tensor_mul(out, *ins)
                elif kind == "tt_add":
                    e_.tensor_add(out, *ins)
                elif kind == "tt_sub":
                    e_.tensor_tensor(out, ins[0], ins[1], mybir.AluOpType.subtract)
                else:
                    e_.tensor_scalar_mul(out, ins[0], scalar)

            def emit_chunk(a, prev8, prevb, s_a, cbase, c0, c1, ht8, htb):
                """One 16-cell chunk: psum prz+pnx (4 banks each, bufs=1).
                Matmuls kc-grouped so sigma/t1/t2 free psum halves early."""
                fd = (c1 - c0) * B
                ptt = ptp.tile([KH, 2, CPB], f8, tag="pt")
                nc.sync.dma_start(
                    out=ptt[:, :, :fd],
                    in_=pt_d[:, :, (cbase + c0) * B:(cbase + c1) * B])
                above = prev8[:, :, (s_a + c0) * B:(s_a + c0) * B + fd]
                left = prev8[:, :, (s_a + 1 + c0) * B:(s_a + 1 + c0) * B + fd]
                above_b = prevb[:, :, (s_a + c0) * B:(s_a + c0) * B + fd]
                left_b = prevb[:, :, (s_a + 1 + c0) * B:(s_a + 1 + c0) * B + fd]

                # P_rz[kc] banks: [r_kc | z_kc]; P_nx[kc]: [nh_kc | xn_kc]
                P_rz = ps.tile([128, 2, 2, CPB], f32, tag="prz")
                P_nx = ps.tile([128, 2, 2, CPB], f32, tag="pnx")
                if not SKIP_MM:
                    for kc in (0, 1):     # r,z gates for hidden chunk kc
                        for mc, g in ((kc, 0), (2 + kc, 1)):
                            csl = slice(mc * 128, (mc + 1) * 128)
                            po = P_rz[:, kc, g, :fd]
                            nc.tensor.matmul(po, wx_sb[a][:, :, csl],
                                             ptt[:, :, :fd], perf_mode=DR,
                                             start=True, stop=False)
                            nc.tensor.matmul(po, uh_sb[a][:, :, csl], above,
                                             perf_mode=DR, start=False,
                                             stop=False)
                            nc.tensor.matmul(po, uh2_sb[a][:, :, csl], left,
                                             perf_mode=DR, start=False,
                                             stop=True)
                    for kc in (0, 1):     # nh (rec only) + xn (input only)
                        mc = 4 + kc
                        csl = slice(mc * 128, (mc + 1) * 128)
                        po = P_nx[:, kc, 0, :fd]
                        nc.tensor.matmul(po, uh_sb[a][:, :, csl], above,
                                         perf_mode=DR, start=True, stop=False)
                        nc.tensor.matmul(po, uh2_sb[a][:, :, csl], left,
                                         perf_mode=DR, start=False, stop=True)
                        nc.tensor.matmul(P_nx[:, kc, 1, :fd],
                                         wx_sb[a][:, :, csl], ptt[:, :, :fd],
                                         perf_mode=DR, start=True, stop=True)

                if SKIP_EW:
                    ob = htb[:, :, (1 + c0) * B:(1 + c0) * B + fd]
                    nc.vector.memset(ob, 0.25)
                    o8 = ht8[:, :, (1 + c0) * B:(1 + c0) * B + fd]
                    nc.gpsimd.memset(o8, 0.25)
                    return
                fe = 2 * fd
                # kc-split psum consumers (free the 4-bank tiles half at a
                # time so the next chunk's matmuls can start early)
                rzt = ew.tile([128, 2, 2, CPB], bf16, tag="rz")
                t1 = ew.tile([128, 2, CPB], bf16, tag="t1")
                t2 = ew.tile([128, 2, CPB], bf16, tag="t2")
                for kc in (0, 1):
                    nc.scalar.activation(rzt[:, kc, :, :fd],
                                         P_rz[:, kc, :, :fd], AF.Sigmoid)
                vop("tt_mul", t1[:, :, :fd], rzt[:, :, 0, :fd],
                    P_nx[:, :, 0, :fd], free=fe, force="v")
                vop("tt_add", t2[:, :, :fd], t1[:, :, :fd],
                    P_nx[:, :, 1, :fd], free=fe, force="v")
                # merged remainder
                nt = ew.tile([128, 2, CPB], bf16, tag="n")
                nc.scalar.activation(nt[:, :, :fd], t2[:, :, :fd], AF.Tanh)
                st = ew.tile([128, 2, CPB], bf16, tag="s")
                vop("tt_add", st[:, :, :fd], above_b, left_b, free=fe)
                dt_ = ew.tile([128, 2, CPB], bf16, tag="d")
                vop("tt_sub", dt_[:, :, :fd], st[:, :, :fd], nt[:, :, :fd],
                    free=fe)
                et = ew.tile([128, 2, CPB], bf16, tag="e")
                vop("tt_mul", et[:, :, :fd], rzt[:, :, 1, :fd], dt_[:, :, :fd],
                    free=fe)
                tmp = ew.tile([128, 2, CPB], bf16, tag="hsum")
                vop("tt_add", tmp[:, :, :fd], et[:, :, :fd], nt[:, :, :fd],
                    free=fe)
                o8 = ht8[:, :, (1 + c0) * B:(1 + c0) * B + fd]
                vop("tsp_mul", o8, tmp[:, :, :fd], free=fe, scalar=1.0)
                ob = htb[:, :, (1 + c0) * B:(1 + c0) * B + fd]
                vop("tsp_mul", ob, tmp[:, :, :fd], free=fe, scalar=0.5)

            # --- main wavefront, 4 directions interleaved per diagonal ---
            max_nd = max(len(di) for di in DIAG_INFOS)
            for _rep in range(REPEAT):
              h_prev = {a: None for a in range(4)}
              for d in range(max_nd):
                 for a in range(4):
                    if d >= len(DIAG_INFOS[a]):
                        continue
                    ilo, ihi, cbase = DIAG_INFOS[a][d]
                    k = ihi - ilo + 1
                    ht8 = hps[a].tile([128, 2, (k + 2) * B], f8, tag=f"h{a}")
                    htb = hps[a].tile([128, 2, (k + 2) * B], bf16, tag=f"hb{a}")
                    for t in (ht8, htb):
                        nc.vector.memset(t[:, :, 0:B], 0.0)
                        nc.gpsimd.memset(t[:, :, (k + 1) * B:(k + 2) * B], 0.0)
                    if d == 0:
                        prev8, prevb, k_prev, ilo_prev = zero_h8, zero_hb, 0, 0
                    else:
                        prev8, prevb, k_prev, ilo_prev = h_prev[a]
                    s_a = ilo - ilo_prev
                    assert 0 <= s_a and s_a + k <= k_prev + 2, (a, d)
                    c0 = 0
                    for cs in _chunk_sizes(k):
                        emit_chunk(a, prev8, prevb, s_a, cbase, c0, c0 + cs,
                                   ht8, htb)
                        c0 += cs
                    h_prev[a] = (ht8, htb, k, ilo)

            # --- head: logits = hcat @ W_out + b_out ; log_softmax ---
            hfin = []
            for a in range(4):
                _, htb, k, _ = h_prev[a]
                assert k == 1
                for kc in (0, 1):
                    t = hd.tile([128, B], f32, tag=f"hf{a}{kc}")
                    nc.scalar.copy(t, htb[:, kc, B:2 * B])
                    hfin.append(t)
            pl_t = ps.tile([128, 2, 2, CPB], f32, tag="prz")
            pl = pl_t[:B, 0, 0, :OUT_DIM]
            for c, t in enumerate(hfin):
                nc.tensor.matmul(pl, t, wo_sb[:, c * OUT_DIM:(c + 1) * OUT_DIM],
                                 start=(c == 0), stop=False)
            nc.tensor.matmul(pl, ones_sb[:1, :B], bo_sb, start=False, stop=True)
            mx = hd.tile([B, 1], f32, tag="mx")
            nc.vector.reduce_max(mx, pl, axis=mybir.AxisListType.X)
            nmx = hd.tile([B, 1], f32, tag="nmx")
            nc.vector.tensor_scalar_mul(nmx, mx, -1.0)
            exv = hd.tile([B, OUT_DIM], f32, tag="exv")
            nc.scalar.activation(exv, pl, AF.Exp, bias=nmx, scale=1.0)
            sm = hd.tile([B, 1], f32, tag="sm")
            nc.vector.reduce_sum(sm, exv, axis=mybir.AxisListType.X)
            lnz = hd.tile([B, 1], f32, tag="lnz")
            nc.scalar.activation(lnz, sm, AF.Ln)
            tot = hd.tile([B, 1], f32, tag="tot")
            nc.vector.tensor_add(tot, lnz, mx)
            ntot = hd.tile([B, 1], f32, tag="ntot")
            nc.vector.tensor_scalar_mul(ntot, tot, -1.0)
            ot = hd.tile([B, OUT_DIM], f32, tag="ot")
            nc.scalar.activation(ot, pl, AF.Identity, bias=ntot, scale=1.0)
            nc.sync.dma_start(out=out_d[:, :], in_=ot)

    _dedupe_ldweights(nc)
    nc.compile()
    return nc


def _dedupe_ldweights(nc):
    """Drop InstLdweights that reload the stationary weights already loaded
    by the immediately preceding load (weight-major groups emit the same
    lhsT for consecutive matmuls). PE executes in order, so the stationary
    contents are still valid; only clean (no-sync) repeats are dropped."""
    ndrop = 0
    for blk in nc.m.functions[0].blocks:
        last_sig = None
        drops = []
        for idx, inst in enumerate(blk.instructions):
            if not str(inst.engine).endswith("PE"):
                continue
            nm = type(inst).__name__
            if nm == "InstLdweights":
                ap = inst.ins[0]
                sig = (getattr(ap, "memref", None), getattr(ap, "offset", None),
                       str(getattr(ap, "ap", None)), str(inst.perf_mode),
                       str(getattr(inst, "tile_position", None)))
                si = inst.sync_info
                clean = si is None or (len(si.on_wait) == 0 and
                                       len(si.on_update) == 0)
                if sig == last_sig and clean:
                    drops.append(idx)
                else:
                    last_sig = sig
            elif nm in ("InstMatmult", "InstEventSemaphore"):
                pass
            else:
                last_sig = None
        for idx in reversed(drops):
            del blk.instructions[idx]
        ndrop += len(drops)
    return ndrop


_CACHE = {}


def get_nc():
    if "nc" not in _CACHE:
        _CACHE["nc"] = _build_nc()
    return _CACHE["nc"]


def make_in_maps(x, Wx, Uh, Uh2, b, W_out, b_out):
    x = np.asarray(x, np.float32)
    wm = make_weight_maps(Wx, Uh, Uh2, b, W_out, b_out)
    in_maps = []
    for c in range(N_CORES):
        xc = x[c * B:(c + 1) * B]
        m = dict(wm)
        m["pt"] = make_pt(xc)
        in_maps.append(m)
    return in_maps


def kernel(x, Wx, Uh, Uh2, b, W_out, b_out):
    from concourse.bass_utils import run_bass_kernel_spmd
    nc = get_nc()
    in_maps = make_in_maps(x, Wx, Uh, Uh2, b, W_out, b_out)
    res = run_bass_kernel_spmd(nc, in_maps, list(range(N_CORES)))
    out = np.concatenate([res.results[c]["out"] for c in range(N_CORES)], axis=0)
    return out.astype(np.float32)


# revision 26
# speedup vs baseline: 1.2113x; 1.0282x over previous
"""MD-RNN (4-direction 2D GRU) Trainium2 kernel, fp8 DoubleRow edition.

Sharding: 8-way data-parallel over batch (B=256 -> 32 per core); each core runs
all 4 directional 2D-GRU scans as anti-diagonal wavefronts, interleaved so all
engines stay busy.

Layout (hidden-on-partition, "transposed"):
  - hidden states h^T: (128 partitions = hidden chunk kc in {0,1}, cells*B free)
    stored twice: fp8e4m3 (matmul operand) and bf16 (skip-path + head).
  - all matmuls are fp8 DoubleRow (2 stacked K=128 k-tiles per instruction):
    per 16-cell chunk, 18 DR instructions (1 per distinct lhsT -- weight
    loads are ~40% of PE time, so each weight is loaded exactly once per
    chunk) fill psum tiles P_rz / P_nx (4 banks each, kc-major so the
    kc-split sigma / t1 / t2 consumers free psum halves early).
  - bf16 h store carries 0.5*h (W_out pre-scaled by 2) so d = s - n is a
    plain tensor_tensor op; DVE/Pool assignment via greedy cost balancing.

The patch tensor (im2col of x + bias ones row, split into two 9-row halves for
DoubleRow) is built host-side and streamed per-chunk from DRAM.
"""

import numpy as np
import ml_dtypes

GRID = 4
N_IMG = 32
S = N_IMG - (GRID - 1)          # 29 patch positions per axis
B_FULL = 256
N_CORES = 8
B = B_FULL // N_CORES           # 32 batch per core
H = 256
H3 = 3 * H                      # 768
OUT_DIM = 10
K_IN = GRID * GRID + 1          # 16 patch elems + ones row (bias trick)
KH = 9                          # DoubleRow half-height of the input projection

FWD = list(range(S))                 # 29 entries
BWD = list(range(S - 2, -1, -1))     # 28 entries (reference off-by-one kept)
DIRS = [(FWD, FWD), (BWD, FWD), (FWD, BWD), (BWD, BWD)]

CELLS_PER_CHUNK = 16            # 16 cells * B=32 = 512 cols; 8 psum banks/chunk
REPEAT = 1                      # body repetitions (timing calibration only)
SKIP_MM = False                 # timing experiment: drop matmuls
SKIP_EW = False                 # timing experiment: drop elementwise gate math

F8 = ml_dtypes.float8_e4m3
BF = ml_dtypes.bfloat16


def _diag_infos():
    """Per direction: list over diagonals of (i_lo, i_hi, global cell base)."""
    infos = []
    base = 0
    for (yi, xi) in DIRS:
        ny, nx = len(yi), len(xi)
        diags = []
        for d in range(ny + nx - 1):
            ilo = max(0, d - (nx - 1))
            ihi = min(d, ny - 1)
            diags.append((ilo, ihi, base))
            base += ihi - ilo + 1
        infos.append(diags)
    return infos, base


DIAG_INFOS, TOT_CELLS = _diag_infos()


def _scan_index_arrays():
    """Image-space (y, x) of every cell in pt order (dir-major, diag-major)."""
    ys, xs = [], []
    for a, (yi, xi) in enumerate(DIRS):
        for d, (ilo, ihi, _) in enumerate(DIAG_INFOS[a]):
            for i in range(ilo, ihi + 1):
                ys.append(yi[i])
                xs.append(xi[d - i])
    return np.asarray(ys), np.asarray(xs)


YS, XS = _scan_index_arrays()


def _chunk_sizes(k):
    nch = (k + CELLS_PER_CHUNK - 1) // CELLS_PER_CHUNK
    lo = k // nch
    rem = k - lo * nch
    return [lo + 1] * rem + [lo] * (nch - rem)


def make_pt(xc):
    """(B, 32, 32) core batch slice -> (9, 2, TOT_CELLS*B) fp8 patch matrix.

    Half 0 = patch rows 0..8; half 1 = rows 9..15, the ones (bias) row, and a
    zero pad row.  DoubleRow sums W0^T@P0 + W1^T@P1 = full 17-row projection.
    """
    from numpy.lib.stride_tricks import sliding_window_view
    w = sliding_window_view(xc, (GRID, GRID), axis=(1, 2))   # (B, 29, 29, 4, 4)
    p = w[:, YS, XS].reshape(xc.shape[0], TOT_CELLS, GRID * GRID)  # (B, T, 16)
    p = np.ascontiguousarray(p.transpose(2, 1, 0)).reshape(GRID * GRID, -1)
    cols = p.shape[1]
    pt = np.zeros((KH, 2, cols), np.float32)
    pt[:, 0] = p[0:KH]
    pt[0:GRID * GRID - KH, 1] = p[KH:]
    pt[GRID * GRID - KH, 1] = 1.0          # bias row
    return np.ascontiguousarray(pt.astype(F8))


def make_weight_maps(Wx, Uh, Uh2, b, W_out, b_out):
    Wx, Uh, Uh2 = (np.asarray(t, np.float32) for t in (Wx, Uh, Uh2))
    b, W_out, b_out = (np.asarray(t, np.float32) for t in (b, W_out, b_out))
    # recurrence weights as DoubleRow lhsT: [a, 128, kc(2), 3H]
    uh = np.empty((4, 128, 2, H3), np.float32)
    uh2 = np.empty((4, 128, 2, H3), np.float32)
    for a in range(4):
        for kc in range(2):
            uh[a, :, kc, :] = Uh[a][kc * 128:(kc + 1) * 128]
            uh2[a, :, kc, :] = Uh2[a][kc * 128:(kc + 1) * 128]
    # input projection as DoubleRow lhsT: [a, 9, half(2), 3H]
    wx = np.zeros((4, KH, 2, H3), np.float32)
    for a in range(4):
        wx[a, :, 0, :] = Wx[a][0:KH]
        wx[a, 0:GRID * GRID - KH, 1, :] = Wx[a][KH:]
        wx[a, GRID * GRID - KH, 1, :] = b[a]
    wo = np.ascontiguousarray(2.0 * W_out.reshape(8, 128, OUT_DIM))
    bo = np.ascontiguousarray(b_out.reshape(1, OUT_DIM))
    return {
        "uh": np.ascontiguousarray(uh.astype(F8)),
        "uh2": np.ascontiguousarray(uh2.astype(F8)),
        "wx": np.ascontiguousarray(wx.astype(F8)),
        "wo": wo,
        "bo": bo,
    }


def _build_nc():
    import concourse.bacc as bacc
    import concourse.mybir as mybir
    import concourse.tile as tile

    f32 = mybir.dt.float32
    f8 = mybir.dt.float8e4
    bf16 = mybir.dt.bfloat16
    AF = mybir.ActivationFunctionType
    ALU = mybir.AluOpType
    DR = mybir.MatmulPerfMode.DoubleRow
    CPB = CELLS_PER_CHUNK * B

    PS_BUFS = 1
    nc = bacc.Bacc("TRN2", target_bir_lowering=False, debug=False,
                   num_devices=N_CORES)
    pt_d = nc.dram_tensor("pt", [KH, 2, TOT_CELLS * B], f8, kind="ExternalInput")
    uh_d = nc.dram_tensor("uh", [4, 128, 2, H3], f8, kind="ExternalInput")
    uh2_d = nc.dram_tensor("uh2", [4, 128, 2, H3], f8, kind="ExternalInput")
    wx_d = nc.dram_tensor("wx", [4, KH, 2, H3], f8, kind="ExternalInput")
    wo_d = nc.dram_tensor("wo", [8, 128, OUT_DIM], f32, kind="ExternalInput")
    bo_d = nc.dram_tensor("bo", [1, OUT_DIM], f32, kind="ExternalInput")
    out_d = nc.dram_tensor("out", [B, OUT_DIM], f32, kind="ExternalOutput")

    with tile.TileContext(nc) as tc:
        from contextlib import ExitStack
        with ExitStack() as ctx:
            const = ctx.enter_context(tc.tile_pool(name="const", bufs=1))
            ptp = ctx.enter_context(tc.tile_pool(name="ptp", bufs=8))
            ps = ctx.enter_context(tc.tile_pool(name="ps", bufs=PS_BUFS, space="PSUM"))
            hps = [ctx.enter_context(tc.tile_pool(name=f"h{a}", bufs=3))
                   for a in range(4)]
            ew = ctx.enter_context(tc.tile_pool(name="ew", bufs=6))
            hd = ctx.enter_context(tc.tile_pool(name="hd", bufs=1))

            # --- resident weights ---
            uh_sb, uh2_sb, wx_sb = {}, {}, {}
            for a in range(4):
                t = const.tile([128, 2, H3], f8, tag=f"uh{a}")
                nc.sync.dma_start(out=t, in_=uh_d[a])
                uh_sb[a] = t
                t = const.tile([128, 2, H3], f8, tag=f"uh2{a}")
                nc.sync.dma_start(out=t, in_=uh2_d[a])
                uh2_sb[a] = t
                t = const.tile([KH, 2, H3], f8, tag=f"wx{a}")
                nc.sync.dma_start(out=t, in_=wx_d[a])
                wx_sb[a] = t
            wo_sb = const.tile([128, 8 * OUT_DIM], f32, tag="wo")
            for c in range(8):
                nc.sync.dma_start(out=wo_sb[:, c * OUT_DIM:(c + 1) * OUT_DIM],
                                  in_=wo_d[c])
            bo_sb = const.tile([1, OUT_DIM], f32, tag="bo")
            nc.sync.dma_start(out=bo_sb, in_=bo_d[:, :])
            ones_sb = const.tile([1, B], f32, tag="ones")
            nc.vector.memset(ones_sb, 1.0)
            zero_h8 = const.tile([128, 2, 2 * B], f8, tag="zeroh8")
            nc.vector.memset(zero_h8, 0.0)
            zero_hb = const.tile([128, 2, 2 * B], bf16, tag="zerohb")
            nc.vector.memset(zero_hb, 0.0)

            # Greedy DVE/Pool balancing: running busy-ns estimate per engine.
            ew_load = {"v": 0.0, "p": 0.0}

            def vop(kind, out, *ins, free, force=None, scalar=None):
                """kind: tt_mul | tt_add | tt_sub | tsp_mul; free: per-lane
                elem count. Picks DVE ('v') or Pool ('p') by projected cost."""
                psum_in = any(str(t.space).endswith("PSUM") for t in ins)
                b1 = all(mybir.dt.size(t.dtype) == 2 for t in (out, *ins))
                if kind == "tsp_mul":
                    cv = free * (0.26 if b1 and not psum_in else 1.042) + 60
                    cp = 1e9          # TensorScalarPtr unsupported on Pool
                    force = force or "v"
                else:
                    cv = (free * (0.521 if (b1 and not psum_in) else 1.042)
                          + (125 if psum_in else 60))
                    cp = free * 1.984 + 95
                    if psum_in:
                        cp = 1e9      # Pool+PSUM untested on walrus
                eng = force or ("v" if ew_load["v"] + cv <= ew_load["p"] + cp
                                else "p")
                e_ = nc.vector if eng == "v" else nc.gpsimd
                ew_load[eng] += cv if eng == "v" else cp
                if kind == "tt_mul":
                    e_.tensor_mul(out, *ins)
                elif kind == "tt_add":
                    e_.tensor_add(out, *ins)
                elif kind == "tt_sub":
                    e_.tensor_tensor(out, ins[0], ins[1], mybir.AluOpType.subtract)
                else:
                    e_.tensor_scalar_mul(out, ins[0], scalar)

            def emit_chunk(a, prev8, prevb, s_a, cbase, c0, c1, ht8, htb):
                """One 16-cell chunk: psum prz+pnx (4 banks each, bufs=1).
                Matmuls kc-grouped so sigma/t1/t2 free psum halves early."""
                fd = (c1 - c0) * B
                ptt = ptp.tile([KH, 2, CPB], f8, tag="pt")
                nc.sync.dma_start(
                    out=ptt[:, :, :fd],
                    in_=pt_d[:, :, (cbase + c0) * B:(cbase + c1) * B])
                above = prev8[:, :, (s_a + c0) * B:(s_a + c0) * B + fd]
                left = prev8[:, :, (s_a + 1 + c0) * B:(s_a + 1 + c0) * B + fd]
                above_b = prevb[:, :, (s_a + c0) * B:(s_a + c0) * B + fd]
                left_b = prevb[:, :, (s_a + 1 + c0) * B:(s_a + 1 + c0) * B + fd]

                # P_rz[kc] banks: [r_kc | z_kc]; P_nx[kc]: [nh_kc | xn_kc]
                P_rz = ps.tile([128, 2, 2, CPB], f32, tag="prz")
                P_nx = ps.tile([128, 2, 2, CPB], f32, tag="pnx")
                if not SKIP_MM:
                    # kc-complete-first order: all 9 matmuls of hidden chunk
                    # kc land before kc+1's, so sigma/t1/t2 of kc overlap the
                    # other half's matmuls.
                    for kc in (0, 1):
                        for mc, g in ((kc, 0), (2 + kc, 1)):
                            csl = slice(mc * 128, (mc + 1) * 128)
                            po = P_rz[:, kc, g, :fd]
                            nc.tensor.matmul(po, wx_sb[a][:, :, csl],
                                             ptt[:, :, :fd], perf_mode=DR,
                                             start=True, stop=False)
                            nc.tensor.matmul(po, uh_sb[a][:, :, csl], above,
                                             perf_mode=DR, start=False,
                                             stop=False)
                            nc.tensor.matmul(po, uh2_sb[a][:, :, csl], left,
                                             perf_mode=DR, start=False,
                                             stop=True)
                        mc = 4 + kc       # nh (rec only) + xn (input only)
                        csl = slice(mc * 128, (mc + 1) * 128)
                        po = P_nx[:, kc, 0, :fd]
                        nc.tensor.matmul(po, uh_sb[a][:, :, csl], above,
                                         perf_mode=DR, start=True, stop=False)
                        nc.tensor.matmul(po, uh2_sb[a][:, :, csl], left,
                                         perf_mode=DR, start=False, stop=True)
                        nc.tensor.matmul(P_nx[:, kc, 1, :fd],
                                         wx_sb[a][:, :, csl], ptt[:, :, :fd],
                                         perf_mode=DR, start=True, stop=True)

                if SKIP_EW:
                    ob = htb[:, :, (1 + c0) * B:(1 + c0) * B + fd]
                    nc.vector.memset(ob, 0.25)
                    o8 = ht8[:, :, (1 + c0) * B:(1 + c0) * B + fd]
                    nc.gpsimd.memset(o8, 0.25)
                    return
                fe = 2 * fd
                # kc-split psum consumers (free the 4-bank tiles half at a
                # time so the next chunk's matmuls can start early)
                rzt = ew.tile([128, 2, 2, CPB], bf16, tag="rz")
                t1 = ew.tile([128, 2, CPB], bf16, tag="t1")
                t2 = ew.tile([128, 2, CPB], bf16, tag="t2")
                for kc in (0, 1):
                    nc.scalar.activation(rzt[:, kc, :, :fd],
                                         P_rz[:, kc, :, :fd], AF.Sigmoid)
                for kc in (0, 1):
                    vop("tt_mul", t1[:, kc, :fd], rzt[:, kc, 0, :fd],
                        P_nx[:, kc, 0, :fd], free=fd, force="v")
                    vop("tt_add", t2[:, kc, :fd], t1[:, kc, :fd],
                        P_nx[:, kc, 1, :fd], free=fd, force="v")
                # merged remainder
                nt = ew.tile([128, 2, CPB], bf16, tag="n")
                nc.scalar.activation(nt[:, :, :fd], t2[:, :, :fd], AF.Tanh)
                st = ew.tile([128, 2, CPB], bf16, tag="s")
                vop("tt_add", st[:, :, :fd], above_b, left_b, free=fe)
                dt_ = ew.tile([128, 2, CPB], bf16, tag="d")
                vop("tt_sub", dt_[:, :, :fd], st[:, :, :fd], nt[:, :, :fd],
                    free=fe)
                et = ew.tile([128, 2, CPB], bf16, tag="e")
                vop("tt_mul", et[:, :, :fd], rzt[:, :, 1, :fd], dt_[:, :, :fd],
                    free=fe)
                tmp = ew.tile([128, 2, CPB], bf16, tag="hsum")
                vop("tt_add", tmp[:, :, :fd], et[:, :, :fd], nt[:, :, :fd],
                    free=fe)
                o8 = ht8[:, :, (1 + c0) * B:(1 + c0) * B + fd]
                vop("tsp_mul", o8, tmp[:, :, :fd], free=fe, scalar=1.0)
                ob = htb[:, :, (1 + c0) * B:(1 + c0) * B + fd]
                vop("tsp_mul", ob, tmp[:, :, :fd], free=fe, scalar=0.5)

            # --- main wavefront, 4 directions interleaved per diagonal ---
            max_nd = max(len(di) for di in DIAG_INFOS)
            for _rep in range(REPEAT):
              h_prev = {a: None for a in range(4)}
              for d in range(max_nd):
                 for a in range(4):
                    if d >= len(DIAG_INFOS[a]):
                        continue
                    ilo, ihi, cbase = DIAG_INFOS[a][d]
                    k = ihi - ilo + 1
                    ht8 = hps[a].tile([128, 2, (k + 2) * B], f8, tag=f"h{a}")
                    htb = hps[a].tile([128, 2, (k + 2) * B], bf16, tag=f"hb{a}")
                    for t in (ht8, htb):
                        nc.vector.memset(t[:, :, 0:B], 0.0)
                        nc.gpsimd.memset(t[:, :, (k + 1) * B:(k + 2) * B], 0.0)
                    if d == 0:
                        prev8, prevb, k_prev, ilo_prev = zero_h8, zero_hb, 0, 0
                    else:
                        prev8, prevb, k_prev, ilo_prev = h_prev[a]
                    s_a = ilo - ilo_prev
                    assert 0 <= s_a and s_a + k <= k_prev + 2, (a, d)
                    c0 = 0
                    for cs in _chunk_sizes(k):
                        emit_chunk(a, prev8, prevb, s_a, cbase, c0, c0 + cs,
                                   ht8, htb)
                        c0 += cs
                    h_prev[a] = (ht8, htb, k, ilo)

            # --- head: logits = hcat @ W_out + b_out ; log_softmax ---
            hfin = []
            for a in range(4):
                _, htb, k, _ = h_prev[a]
                assert k == 1
                for kc in (0, 1):
                    t = hd.tile([128, B], f32, tag=f"hf{a}{kc}")
                    nc.scalar.copy(t, htb[:, kc, B:2 * B])
                    hfin.append(t)
            pl_t = ps.tile([128, 2, 2, CPB], f32, tag="prz")
            pl = pl_t[:B, 0, 0, :OUT_DIM]
            for c, t in enumerate(hfin):
                nc.tensor.matmul(pl, t, wo_sb[:, c * OUT_DIM:(c + 1) * OUT_DIM],
                                 start=(c == 0), stop=False)
            nc.tensor.matmul(pl, ones_sb[:1, :B], bo_sb, start=False, stop=True)
            mx = hd.tile([B, 1], f32, tag="mx")
            nc.vector.reduce_max(mx, pl, axis=mybir.AxisListType.X)
            nmx = hd.tile([B, 1], f32, tag="nmx")
            nc.vector.tensor_scalar_mul(nmx, mx, -1.0)
            exv = hd.tile([B, OUT_DIM], f32, tag="exv")
            nc.scalar.activation(exv, pl, AF.Exp, bias=nmx, scale=1.0)
            sm = hd.tile([B, 1], f32, tag="sm")
            nc.vector.reduce_sum(sm, exv, axis=mybir.AxisListType.X)
            lnz = hd.tile([B, 1], f32, tag="lnz")
            nc.scalar.activation(lnz, sm, AF.Ln)
            tot = hd.tile([B, 1], f32, tag="tot")
            nc.vector.tensor_add(tot, lnz, mx)
            ntot = hd.tile([B, 1], f32, tag="ntot")
            nc.vector.tensor_scalar_mul(ntot, tot, -1.0)
            ot = hd.tile([B, OUT_DIM], f32, tag="ot")
            nc.scalar.activation(ot, pl, AF.Identity, bias=ntot, scale=1.0)
            nc.sync.dma_start(out=out_d[:, :], in_=ot)

    _dedupe_ldweights(nc)
    nc.compile()
    return nc


def _dedupe_ldweights(nc):
    """Drop InstLdweights that reload the stationary weights already loaded
    by the immediately preceding load (weight-major groups emit the same
    lhsT for consecutive matmuls). PE executes in order, so the stationary
    contents are still valid; only clean (no-sync) repeats are dropped."""
    ndrop = 0
    for blk in nc.m.functions[0].blocks:
        last_sig = None
        drops = []
        for idx, inst in enumerate(blk.instructions):
            if not str(inst.engine).endswith("PE"):
                continue
            nm = type(inst).__name__
            if nm == "InstLdweights":
                ap = inst.ins[0]
                sig = (getattr(ap, "memref", None), getattr(ap, "offset", None),
                       str(getattr(ap, "ap", None)), str(inst.perf_mode),
                       str(getattr(inst, "tile_position", None)))
                si = inst.sync_info
                clean = si is None or (len(si.on_wait) == 0 and
                                       len(si.on_update) == 0)
                if sig == last_sig and clean:
                    drops.append(idx)
                else:
                    last_sig = sig
            elif nm in ("InstMatmult", "InstEventSemaphore"):
                pass
            else:
                last_sig = None
        for idx in reversed(drops):
            del blk.instructions[idx]
        ndrop += len(drops)
    return ndrop


_CACHE = {}


def get_nc():
    if "nc" not in _CACHE:
        _CACHE["nc"] = _build_nc()
    return _CACHE["nc"]


def make_in_maps(x, Wx, Uh, Uh2, b, W_out, b_out):
    x = np.asarray(x, np.float32)
    wm = make_weight_maps(Wx, Uh, Uh2, b, W_out, b_out)
    in_maps = []
    for c in range(N_CORES):
        xc = x[c * B:(c + 1) * B]
        m = dict(wm)
        m["pt"] = make_pt(xc)
        in_maps.append(m)
    return in_maps


def kernel(x, Wx, Uh, Uh2, b, W_out, b_out):
    from concourse.bass_utils import run_bass_kernel_spmd
    nc = get_nc()
    in_maps = make_in_maps(x, Wx, Uh, Uh2, b, W_out, b_out)
    res = run_bass_kernel_spmd(nc, in_maps, list(range(N_CORES)))
    out = np.concatenate([res.results[c]["out"] for c in range(N_CORES)], axis=0)
    return out.astype(np.float32)
